# revision 14
# baseline (speedup 1.0000x reference)
"""Trainium2 Bass kernel for nn_CrossModalBlock (cross-attention transformer block).

Sharding: data-parallel over batch B=8 across the 8 NeuronCores (one batch
element per core, weights replicated). No collectives.

Device-side layout strategy: activations are kept in "transposed" layout
[features, tokens] throughout, so every linear layer's contraction dim (the
feature dim) is on SBUF partitions for both operands and no on-device
transposes are ever needed.  Softmax runs along the partition (text-token)
axis: denominators come from an extra ones-column appended to V, and the
reciprocal row is broadcast across partitions with GpSimd partition_broadcast.
Attention probabilities are written out transposed [h, n, p] and fixed up on
the host during unshard (pure layout marshaling).

SBUF is managed as one flat set of pools with tag-based slot-reuse chains
(e.g. the slot holding imgT is later reused for part of the FFN hidden
activations) so the whole kernel fits without pool scoping.
"""

import os
import math
import numpy as np

import concourse.bass as bass
import concourse.mybir as mybir
from concourse import bacc
from concourse.tile import TileContext
from concourse.bass_utils import run_bass_kernel_spmd

# problem dims (hardcoded per spec)
D = 1024
H = 16
DH = 64          # head dim
HID = 4096
B = 8
P = 1024         # img tokens
N = 512          # txt tokens
EPS = 1e-5
N_CORES = 8

F32 = mybir.dt.float32

# matmul input dtype: float32r streams fp32 data through the PE at full
# (1 cycle/row) rate; float32 is exact but 4 cycles/row.
_MMDT_NAME = os.environ.get("BASSK_MMDT", "float32r")

AF = mybir.ActivationFunctionType
ALU = mybir.AluOpType


def _build_program(mmdt_name=None, repeat=1):
    mmdt = getattr(mybir.dt, mmdt_name or _MMDT_NAME)
    nc = bacc.Bacc(None, target_bir_lowering=False, debug=False)

    # ---- per-core DRAM inputs ----
    imgT_d = nc.dram_tensor("imgT", [D, P], mmdt, kind="ExternalInput")
    txtT_d = nc.dram_tensor("txtT", [D, N], mmdt, kind="ExternalInput")
    wqT_d = nc.dram_tensor("wqT", [D, D], mmdt, kind="ExternalInput")
    wkT_d = nc.dram_tensor("wkT", [D, D], mmdt, kind="ExternalInput")
    wvT_d = nc.dram_tensor("wvT", [D, D], mmdt, kind="ExternalInput")
    woT_d = nc.dram_tensor("woT", [D, D], mmdt, kind="ExternalInput")
    w1T_d = nc.dram_tensor("w1T", [D, HID], mmdt, kind="ExternalInput")
    w2T_d = nc.dram_tensor("w2T", [HID, D], mmdt, kind="ExternalInput")
    bq_d = nc.dram_tensor("bq", [128, 8], F32, kind="ExternalInput")
    bk_d = nc.dram_tensor("bk", [128, 8], F32, kind="ExternalInput")
    bv_d = nc.dram_tensor("bv", [128, 8], F32, kind="ExternalInput")
    bo_d = nc.dram_tensor("bo", [128, 8], F32, kind="ExternalInput")
    b1_d = nc.dram_tensor("b1", [128, 32], F32, kind="ExternalInput")
    b2_d = nc.dram_tensor("b2", [128, 8], F32, kind="ExternalInput")
    cb_d = nc.dram_tensor("cb", [1, 1], F32, kind="ExternalInput")
    # [128, x] per-partition marshaled vectors
    ltau_d = nc.dram_tensor("ltau", [128, 8], F32, kind="ExternalInput")
    maskb_d = nc.dram_tensor("maskb", [128, 4], F32, kind="ExternalInput")
    clsw_d = nc.dram_tensor("clsw", [128, 8], F32, kind="ExternalInput")
    g1_d = nc.dram_tensor("g1", [128, 8], F32, kind="ExternalInput")
    gb1_d = nc.dram_tensor("gb1", [128, 8], F32, kind="ExternalInput")
    g2_d = nc.dram_tensor("g2", [128, 8], F32, kind="ExternalInput")
    gb2_d = nc.dram_tensor("gb2", [128, 8], F32, kind="ExternalInput")

    # ---- per-core DRAM outputs ----
    xT_o = nc.dram_tensor("xT_o", [D, P], F32, kind="ExternalOutput")
    probsT_o = nc.dram_tensor("probsT_o", [H, N, P], F32, kind="ExternalOutput")
    logits_o = nc.dram_tensor("logits_o", [1, P], F32, kind="ExternalOutput")
    sig_o = nc.dram_tensor("sig_o", [1, P], F32, kind="ExternalOutput")

    def mm(ps, lhsT, rhs, start, stop):
        nc.tensor.matmul(ps, lhsT, rhs, start=start, stop=stop)

    def f32(ap):
        return ap.bitcast(F32)

    with TileContext(nc) as tc:
        with (
            tc.tile_pool(name="const", bufs=1) as cpool,
            tc.tile_pool(name="ps", bufs=8, space="PSUM") as pspool,
            tc.tile_pool(name="rows", bufs=5) as rpool,
            tc.tile_pool(name="work", bufs=5) as tpool,
            tc.tile_pool(name="wbig", bufs=3) as wpool,
            tc.tile_pool(name="big", bufs=1) as gpool,
        ):
          def emit_body():
            def ctile(shape, tag):
                return cpool.tile(shape, F32, tag=tag, name=tag)

            def wtile(name):
                return wpool.tile([128, 8, 256], mmdt, tag="w", name=name)

            # ---------- constants / small inputs ----------
            ones_col = ctile([128, 1], "ones_col")
            nc.vector.memset(ones_col[:], 1.0)
            ones_colr = cpool.tile([128, 1], mmdt, tag="ones_colr", name="ones_colr")
            nc.scalar.activation(ones_colr[:], ones_col[:], AF.Copy)
            ones_row = ctile([1, 128], "ones_row")
            nc.vector.memset(ones_row[:], 1.0)
            ones_rowr = cpool.tile([1, 128], mmdt, tag="ones_rowr", name="ones_rowr")
            nc.scalar.activation(ones_rowr[:], ones_row[:], AF.Copy)

            ln8 = ctile([128, 1], "ln8")
            nc.vector.memset(ln8[:], float(math.log(0.125)))
            epsrow = ctile([1, 1], "epsrow")
            nc.vector.memset(epsrow[:], EPS)

            svec = ctile([128, 8], "svec")
            nc.sync.dma_start(out=svec[:], in_=ltau_d[:])
            # svec = exp(-log_tau) / 8   (folds 1/sqrt(dh)=1/8 and 1/tau)
            nc.scalar.activation(svec[:], svec[:], AF.Exp,
                                 scale=-1.0, bias=ln8[:])

            small_loads = [("maskb", maskb_d, [128, 4]), ("clsw", clsw_d, [128, 8]),
                           ("g1", g1_d, [128, 8]), ("gb1", gb1_d, [128, 8]),
                           ("g2", g2_d, [128, 8]), ("gb2", gb2_d, [128, 8]),
                           ("bqp", bq_d, [128, 8]), ("bkp", bk_d, [128, 8]),
                           ("bvp", bv_d, [128, 8]), ("bop", bo_d, [128, 8]),
                           ("b1p", b1_d, [128, 32]), ("b2p", b2_d, [128, 8]),
                           ("cbr", cb_d, [1, 1])]
            sm = {}
            for nm, dram, shp in small_loads:
                t = ctile(shp, nm)
                nc.sync.dma_start(out=t[:], in_=dram[:])
                sm[nm] = t
            maskb, clsw = sm["maskb"], sm["clsw"]
            g1, gb1, g2, gb2 = sm["g1"], sm["gb1"], sm["g2"], sm["gb2"]
            bqp, bkp, bvp, bop = sm["bqp"], sm["bkp"], sm["bvp"], sm["bop"]
            b1p, b2p, cbr = sm["b1p"], sm["b2p"], sm["cbr"]
            clswr = cpool.tile([128, 8], mmdt, tag="clswr", name="clswr")
            nc.scalar.activation(clswr[:], clsw[:], AF.Copy)
            # q bias must be pre-scaled by svec (drain computes ps*svec + bias)
            bqs = ctile([128, 8], "bqs")
            nc.vector.tensor_mul(bqs[:], bqp[:], svec[:])

            def ln_transposed(xt, g_t, gb_t, xdt):
                """In-place LayerNorm over the feature axis (partition x
                po-chunk) of a [128, 8, 1024] transposed activation tile."""
                oc = ones_colr if xdt is not F32 else ones_col
                for pc in range(2):
                    pcs = slice(pc * N, (pc + 1) * N)
                    sum_ps = pspool.tile([128, N], F32, tag="ps", name="sum_ps")
                    for dc in range(8):
                        mm(sum_ps[0:1, :], oc[:], xt[:, dc, pcs],
                           start=(dc == 0), stop=(dc == 7))
                    sumsq_ps = pspool.tile([128, N], F32, tag="ps", name="sumsq_ps")
                    for dc in range(8):
                        sq = tpool.tile([128, N], xdt, tag="work", name="sq")
                        nc.scalar.activation(sq[:], f32(xt[:, dc, pcs]), AF.Square)
                        mm(sumsq_ps[0:1, :], oc[:], sq[:],
                           start=(dc == 0), stop=(dc == 7))
                    mu = rpool.tile([1, N], F32, tag="rows", name="mu")
                    nc.scalar.activation(mu[:], sum_ps[0:1, :], AF.Copy,
                                         scale=1.0 / D)
                    ex2 = rpool.tile([1, N], F32, tag="rows", name="ex2")
                    nc.scalar.activation(ex2[:], sumsq_ps[0:1, :], AF.Copy,
                                         scale=1.0 / D)
                    var = rpool.tile([1, N], F32, tag="rows", name="var")
                    nc.scalar.activation(var[:], mu[:], AF.Square)
                    nc.vector.tensor_sub(var[:], ex2[:], var[:])
                    std = rpool.tile([1, N], F32, tag="rows", name="std")
                    nc.scalar.activation(std[:], var[:], AF.Sqrt, bias=epsrow[:])
                    rstd = rpool.tile([1, N], F32, tag="rows", name="rstd")
                    nc.vector.reciprocal(rstd[:], std[:])
                    mu_bc = tpool.tile([128, N], F32, tag="work", name="mu_bc")
                    nc.gpsimd.partition_broadcast(mu_bc[:], mu[:])
                    rstd_bc = tpool.tile([128, N], F32, tag="work", name="rstd_bc")
                    nc.gpsimd.partition_broadcast(rstd_bc[:], rstd[:])
                    for dc in range(8):
                        t1 = tpool.tile([128, N], F32, tag="work", name="t1")
                        nc.vector.tensor_sub(t1[:], f32(xt[:, dc, pcs]), mu_bc[:])
                        nc.vector.tensor_mul(t1[:], t1[:], rstd_bc[:])
                        nc.vector.tensor_scalar(
                            out=xt[:, dc, pcs], in0=t1[:],
                            scalar1=g_t[:, dc:dc + 1], scalar2=gb_t[:, dc:dc + 1],
                            op0=ALU.mult, op1=ALU.add)

            # ================= Phase A: QKV projections =================
            imgT = gpool.tile([128, 8, P], mmdt, tag="bigA", name="imgT")
            nc.sync.dma_start(
                out=imgT[:], in_=imgT_d[:].rearrange("(po pi) p -> pi po p", pi=128))
            txtT = gpool.tile([128, 8, N], mmdt, tag="bigD", name="txtT", bufs=2)
            nc.sync.dma_start(
                out=txtT[:], in_=txtT_d[:].rearrange("(po pi) n -> pi po n", pi=128))

            qT = gpool.tile([128, 8, P], mmdt, tag="bigB", name="qT")
            kT = gpool.tile([128, 8, N], mmdt, tag="bigE", name="kT")
            v_sb = gpool.tile([128, 4, H * (DH + 1)], mmdt, tag="bigF", name="v_sb")

            # ones column for every head slot in v (denominator trick)
            nc.vector.tensor_copy(
                v_sb[:].rearrange("q n (h e) -> q n h e", e=DH + 1)[:, :, :, DH:DH + 1],
                ones_col[:].to_broadcast((128, 4, H, 1)))

            # qT = (Wq @ img^T + bq) scaled by svec (per-feature 1/(8*tau_h))
            for dq in range(4):
                dos = slice(256 * dq, 256 * (dq + 1))
                wq_q = wtile(f"wq{dq}")
                nc.sync.dma_start(
                    out=wq_q[:], in_=wqT_d[:, dos].rearrange("(po pi) d -> pi po d", pi=128))
                for dl in range(2):
                    doutc = 2 * dq + dl
                    for pc in range(2):
                        pcs = slice(pc * N, (pc + 1) * N)
                        ps = pspool.tile([128, N], F32, tag="ps", name="ps")
                        for dinc in range(8):
                            mm(ps[:], wq_q[:, dinc, 128 * dl:128 * (dl + 1)],
                               imgT[:, dinc, pcs], start=(dinc == 0), stop=(dinc == 7))
                        nc.scalar.activation(qT[:, doutc, pcs], ps[:], AF.Identity,
                                             scale=svec[:, doutc:doutc + 1],
                                             bias=bqs[:, doutc:doutc + 1])

            # kT = Wk @ txt^T + bk
            for dq in range(4):
                dos = slice(256 * dq, 256 * (dq + 1))
                wk_q = wtile(f"wk{dq}")
                nc.sync.dma_start(
                    out=wk_q[:], in_=wkT_d[:, dos].rearrange("(po pi) d -> pi po d", pi=128))
                for dl in range(2):
                    doutc = 2 * dq + dl
                    ps = pspool.tile([128, N], F32, tag="ps", name="ps")
                    for dinc in range(8):
                        mm(ps[:], wk_q[:, dinc, 128 * dl:128 * (dl + 1)],
                           txtT[:, dinc, :], start=(dinc == 0), stop=(dinc == 7))
                    nc.scalar.activation(kT[:, doutc, :], ps[:], AF.Identity,
                                         bias=bkp[:, doutc:doutc + 1])

            # v natural [n, dout], written into the strided head+1 layout
            for dq in range(4):
                wv_q = wtile(f"wv{dq}")
                nc.sync.dma_start(
                    out=wv_q[:], in_=wvT_d[:, 256 * dq:256 * (dq + 1)].rearrange(
                        "(po pi) d -> pi po d", pi=128))
                for nc4 in range(4):
                    ps = pspool.tile([128, 256], F32, tag="ps", name="ps")
                    for dinc in range(8):
                        mm(ps[:], txtT[:, dinc, 128 * nc4:128 * (nc4 + 1)],
                           wv_q[:, dinc, :], start=(dinc == 0), stop=(dinc == 7))
                    dst = v_sb[:].rearrange("q n (h e) -> q n h e", e=DH + 1)[
                        :, nc4, 4 * dq:4 * (dq + 1), 0:DH]
                    src = ps[:].rearrange("q (h d) -> q h d", d=DH)
                    nc.vector.tensor_copy(dst, src)

            # ================= Phase B: attention =================
            attnT = gpool.tile([128, 8, P], mmdt, tag="bigC", name="attnT")
            for h in range(H):
                hp = 64 * (h % 2)
                po = h // 2
                expT = gpool.tile([128, 4, P], mmdt, tag="bigD", name="expT", bufs=2)
                for pc in range(2):
                    pcs = slice(pc * N, (pc + 1) * N)
                    for nc4 in range(4):
                        sps = pspool.tile([128, N], F32, tag="ps", name="sps")
                        mm(sps[:],
                           kT[hp:hp + 64, po, 128 * nc4:128 * (nc4 + 1)],
                           qT[hp:hp + 64, po, pcs],
                           start=True, stop=True)
                        nc.scalar.activation(expT[:, nc4, pcs], sps[:], AF.Exp,
                                             bias=maskb[:, nc4:nc4 + 1])
                    aps = pspool.tile([128, N], F32, tag="ps", name="aps")
                    for nc4 in range(4):
                        mm(aps[0:DH + 1, :],
                           v_sb[:, nc4, (DH + 1) * h:(DH + 1) * (h + 1)],
                           expT[:, nc4, pcs],
                           start=(nc4 == 0), stop=(nc4 == 3))
                    r_row = rpool.tile([1, N], F32, tag="rows", name="r_row")
                    nc.vector.reciprocal(r_row[:], aps[DH:DH + 1, :])
                    rb = tpool.tile([128, N], F32, tag="work", name="rb")
                    nc.gpsimd.partition_broadcast(rb[:], r_row[:])
                    # normalized attention output (transposed layout) + v bias
                    nc.vector.tensor_mul(attnT[hp:hp + 64, po, pcs],
                                         aps[0:DH, :], rb[0:DH, :])
                    nc.vector.tensor_scalar_add(attnT[hp:hp + 64, po, pcs],
                                                f32(attnT[hp:hp + 64, po, pcs]),
                                                bvp[hp:hp + 64, po:po + 1])
                    # normalized probabilities -> DRAM (transposed)
                    for nc4 in range(4):
                        pt = tpool.tile([128, N], F32, tag="work", name="pt")
                        eng = nc.vector if nc4 % 2 == 0 else nc.gpsimd
                        eng.tensor_mul(pt[:], f32(expT[:, nc4, pcs]), rb[:])
                        nc.sync.dma_start(
                            out=probsT_o[h, 128 * nc4:128 * (nc4 + 1), pcs],
                            in_=pt[:])

            # ============== Phase C: out-proj + LN1 ==============
            x1T = gpool.tile([128, 8, P], mmdt, tag="bigB", name="x1T")
            for dq in range(4):
                wo_q = wtile(f"wo{dq}")
                nc.sync.dma_start(
                    out=wo_q[:], in_=woT_d[:, 256 * dq:256 * (dq + 1)].rearrange(
                        "(po pi) d -> pi po d", pi=128))
                for dl in range(2):
                    doutc = 2 * dq + dl
                    for pc in range(2):
                        pcs = slice(pc * N, (pc + 1) * N)
                        ps = pspool.tile([128, N], F32, tag="ps", name="ps")
                        for dinc in range(8):
                            mm(ps[:], wo_q[:, dinc, 128 * dl:128 * (dl + 1)],
                               attnT[:, dinc, pcs],
                               start=(dinc == 0), stop=(dinc == 7))
                        nc.vector.scalar_tensor_tensor(
                            out=x1T[:, doutc, pcs], in0=ps[:],
                            scalar=bop[:, doutc:doutc + 1],
                            in1=f32(imgT[:, doutc, pcs]),
                            op0=ALU.add, op1=ALU.add)
            ln_transposed(x1T, g1, gb1, mmdt)

            # ============== Phase D: FFN + LN2 ==============
            xT_sb = gpool.tile([128, 8, P], mmdt, tag="bigC", name="xT_sb")
            hT_a = gpool.tile([128, 16, N], mmdt, tag="bigA", name="hT_a")
            hT_b = gpool.tile([128, 8, N], mmdt, tag="bigE", name="hT_b")
            hT_c = gpool.tile([128, 8, N], mmdt, tag="bigF", name="hT_c")

            def h_slot(hc):
                if hc < 16:
                    return hT_a[:, hc, :]
                if hc < 24:
                    return hT_b[:, hc - 16, :]
                return hT_c[:, hc - 24, :]

            for pc in range(2):
                pcs = slice(pc * N, (pc + 1) * N)
                # D1: hT = relu(W1 @ x1^T + b1) for this p-half
                for hc in range(32):
                    w1s = wpool.tile([128, 8, 128], mmdt, tag="w", name="w1s")
                    nc.sync.dma_start(
                        out=w1s[:],
                        in_=w1T_d[:, 128 * hc:128 * (hc + 1)].rearrange(
                            "(po pi) hh -> pi po hh", pi=128))
                    ps = pspool.tile([128, N], F32, tag="ps", name="ps")
                    for dinc in range(8):
                        mm(ps[:], w1s[:, dinc, :], x1T[:, dinc, pcs],
                           start=(dinc == 0), stop=(dinc == 7))
                    nc.scalar.activation(h_slot(hc), ps[:], AF.Relu,
                                         bias=b1p[:, hc:hc + 1])
                # D2: x2 = W2 @ hT + b2 (+x1 residual)
                aps2 = [pspool.tile([128, N], F32, tag="ps", name=f"acc{i}")
                        for i in range(8)]
                for hc in range(32):
                    w2s = wpool.tile([128, 1024], mmdt, tag="w", name="w2s")
                    nc.sync.dma_start(
                        out=w2s[:], in_=w2T_d[128 * hc:128 * (hc + 1), :])
                    for dc in range(8):
                        mm(aps2[dc][:], w2s[:, 128 * dc:128 * (dc + 1)],
                           h_slot(hc), start=(hc == 0), stop=(hc == 31))
                for dc in range(8):
                    nc.vector.scalar_tensor_tensor(
                        out=xT_sb[:, dc, pcs], in0=aps2[dc][:],
                        scalar=b2p[:, dc:dc + 1], in1=f32(x1T[:, dc, pcs]),
                        op0=ALU.add, op1=ALU.add)
            ln_transposed(xT_sb, g2, gb2, mmdt)
            for dc in range(8):
                nc.sync.dma_start(out=xT_o[128 * dc:128 * (dc + 1), :],
                                  in_=f32(xT_sb[:, dc, :]))

            # ============== Phase E: classifier head ==============
            for pc in range(2):
                pcs = slice(pc * N, (pc + 1) * N)
                lp = pspool.tile([128, N], F32, tag="ps", name="lp")
                for dc in range(8):
                    mm(lp[0:1, :], clswr[:, dc:dc + 1], xT_sb[:, dc, pcs],
                       start=(dc == 0), stop=(dc == 7))
                lrow = tpool.tile([128, N], F32, tag="work", name="lrow")
                nc.scalar.activation(lrow[0:1, :], lp[0:1, :], AF.Identity,
                                     bias=cbr[:])
                srow = tpool.tile([128, N], F32, tag="work", name="srow")
                nc.scalar.activation(srow[0:1, :], lrow[0:1, :], AF.Sigmoid)
                nc.sync.dma_start(out=logits_o[0:1, pcs], in_=lrow[0:1, :])
                nc.sync.dma_start(out=sig_o[0:1, pcs], in_=srow[0:1, :])

          if repeat == 1:
              emit_body()
          else:
              with tc.For_i(0, repeat, 1):
                  emit_body()

    nc.finalize()
    return nc


_NC_CACHE = {}


def _get_program(repeat=1):
    key = (_MMDT_NAME, repeat)
    if key not in _NC_CACHE:
        _NC_CACHE[key] = _build_program(_MMDT_NAME, repeat)
    return _NC_CACHE[key]


def make_in_maps(img_emb, txt_emb, text_mask, in_proj_w, in_proj_b, out_w, out_b,
                 log_tau, n1_g, n1_b, ffn_w1, ffn_b1, ffn_w2, ffn_b2, n2_g, n2_b,
                 cls_w, cls_b):
    """Host-side marshaling: shard over batch and lay tensors out as the
    device program expects (all pure transpose/reshape/replication)."""
    f = np.float32
    img_emb = np.asarray(img_emb, f)
    txt_emb = np.asarray(txt_emb, f)
    text_mask = np.asarray(text_mask)
    in_proj_w = np.asarray(in_proj_w, f)
    in_proj_b = np.asarray(in_proj_b, f)

    def pp(vec, cols):
        return np.ascontiguousarray(np.asarray(vec, f).reshape(cols, 128).T)

    shared = {
        "wqT": np.ascontiguousarray(in_proj_w[:D].T),
        "wkT": np.ascontiguousarray(in_proj_w[D:2 * D].T),
        "wvT": np.ascontiguousarray(in_proj_w[2 * D:].T),
        "woT": np.ascontiguousarray(np.asarray(out_w, f).T),
        "w1T": np.ascontiguousarray(np.asarray(ffn_w1, f).T),
        "w2T": np.ascontiguousarray(np.asarray(ffn_w2, f).T),
        "bq": pp(in_proj_b[:D], 8),
        "bk": pp(in_proj_b[D:2 * D], 8),
        "bv": pp(in_proj_b[2 * D:], 8),
        "bo": pp(out_b, 8),
        "b1": pp(ffn_b1, 32),
        "b2": pp(ffn_b2, 8),
        "cb": np.asarray(cls_b, f).reshape(1, 1),
        "ltau": np.ascontiguousarray(
            np.repeat(np.asarray(log_tau, f).reshape(8, 2).T, 64, axis=0)),
        "clsw": pp(np.asarray(cls_w, f)[0], 8),
        "g1": pp(n1_g, 8),
        "gb1": pp(n1_b, 8),
        "g2": pp(n2_g, 8),
        "gb2": pp(n2_b, 8),
    }
    in_maps = []
    for b in range(B):
        mb = np.where(text_mask[b], 0.0, -1e30).astype(f)
        m = dict(shared)
        m["imgT"] = np.ascontiguousarray(img_emb[b].T)
        m["txtT"] = np.ascontiguousarray(txt_emb[b].T)
        m["maskb"] = np.ascontiguousarray(mb.reshape(4, 128).T)
        in_maps.append(m)
    return in_maps


def unshard(results):
    """Gather per-core outputs into full-shape numpy arrays (host-side
    transpose undoes the device's feature-major layout)."""
    x = np.stack([np.ascontiguousarray(r["xT_o"].T) for r in results])
    weights = np.stack(
        [np.ascontiguousarray(r["probsT_o"].transpose(0, 2, 1)) for r in results])
    logits = np.stack([r["logits_o"][0] for r in results])
    sig = np.stack([r["sig_o"][0] for r in results])
    return x, weights, logits, sig


def kernel(**inputs):
    nc = _get_program()
    in_maps = make_in_maps(**inputs)
    res = run_bass_kernel_spmd(nc, in_maps, list(range(N_CORES)))
    return unshard(res.results)


# revision 15
# speedup vs baseline: 1.3946x; 1.3946x over previous
"""Trainium2 Bass kernel for nn_CrossModalBlock (cross-attention transformer block).

Sharding: data-parallel over batch B=8 across the 8 NeuronCores (one batch
element per core, weights replicated). No collectives.

Device-side layout strategy: activations are kept in "transposed" layout
[features, tokens] throughout, so every linear layer's contraction dim (the
feature dim) is on SBUF partitions for both operands and no on-device
transposes are ever needed.  Softmax runs along the partition (text-token)
axis: denominators come from an extra ones-column appended to V, and the
reciprocal row is broadcast across partitions with GpSimd partition_broadcast.
Attention probabilities are written out transposed [h, n, p] and fixed up on
the host during unshard (pure layout marshaling).

SBUF is managed as one flat set of pools with tag-based slot-reuse chains
(e.g. the slot holding imgT is later reused for part of the FFN hidden
activations) so the whole kernel fits without pool scoping.
"""

import os
import math
import numpy as np

import concourse.bass as bass
import concourse.mybir as mybir
from concourse import bacc
from concourse.tile import TileContext
from concourse.bass_utils import run_bass_kernel_spmd

# problem dims (hardcoded per spec)
D = 1024
H = 16
DH = 64          # head dim
HID = 4096
B = 8
P = 1024         # img tokens
N = 512          # txt tokens
EPS = 1e-5
N_CORES = 8

F32 = mybir.dt.float32

# matmul input dtype: float32r streams fp32 data through the PE at full
# (1 cycle/row) rate; float32 is exact but 4 cycles/row.
_MMDT_NAME = os.environ.get("BASSK_MMDT", "float32r")

AF = mybir.ActivationFunctionType
ALU = mybir.AluOpType


def _build_program(mmdt_name=None, repeat=1):
    mmdt = getattr(mybir.dt, mmdt_name or _MMDT_NAME)
    nc = bacc.Bacc(None, target_bir_lowering=False, debug=False)

    # ---- per-core DRAM inputs ----
    imgT_d = nc.dram_tensor("imgT", [D, P], mmdt, kind="ExternalInput")
    txtT_d = nc.dram_tensor("txtT", [D, N], mmdt, kind="ExternalInput")
    wqT_d = nc.dram_tensor("wqT", [D, D], mmdt, kind="ExternalInput")
    wkT_d = nc.dram_tensor("wkT", [D, D], mmdt, kind="ExternalInput")
    wvT_d = nc.dram_tensor("wvT", [D, D], mmdt, kind="ExternalInput")
    woT_d = nc.dram_tensor("woT", [D, D], mmdt, kind="ExternalInput")
    w1T_d = nc.dram_tensor("w1T", [D, HID], mmdt, kind="ExternalInput")
    w2T_d = nc.dram_tensor("w2T", [HID, D], mmdt, kind="ExternalInput")
    bq_d = nc.dram_tensor("bq", [128, 8], F32, kind="ExternalInput")
    bk_d = nc.dram_tensor("bk", [128, 8], F32, kind="ExternalInput")
    bv_d = nc.dram_tensor("bv", [128, 8], F32, kind="ExternalInput")
    bo_d = nc.dram_tensor("bo", [128, 8], F32, kind="ExternalInput")
    b1_d = nc.dram_tensor("b1", [128, 32], F32, kind="ExternalInput")
    b2_d = nc.dram_tensor("b2", [128, 8], F32, kind="ExternalInput")
    cb_d = nc.dram_tensor("cb", [1, 1], F32, kind="ExternalInput")
    # [128, x] per-partition marshaled vectors
    ltau_d = nc.dram_tensor("ltau", [128, 8], F32, kind="ExternalInput")
    maskb_d = nc.dram_tensor("maskb", [128, 4], F32, kind="ExternalInput")
    clsw_d = nc.dram_tensor("clsw", [128, 8], F32, kind="ExternalInput")
    g1_d = nc.dram_tensor("g1", [128, 8], F32, kind="ExternalInput")
    gb1_d = nc.dram_tensor("gb1", [128, 8], F32, kind="ExternalInput")
    g2_d = nc.dram_tensor("g2", [128, 8], F32, kind="ExternalInput")
    gb2_d = nc.dram_tensor("gb2", [128, 8], F32, kind="ExternalInput")

    # ---- per-core DRAM outputs ----
    xT_o = nc.dram_tensor("xT_o", [D, P], F32, kind="ExternalOutput")
    probsT_o = nc.dram_tensor("probsT_o", [H, N, P], F32, kind="ExternalOutput")
    logits_o = nc.dram_tensor("logits_o", [1, P], F32, kind="ExternalOutput")
    sig_o = nc.dram_tensor("sig_o", [1, P], F32, kind="ExternalOutput")

    def mm(ps, lhsT, rhs, start, stop):
        nc.tensor.matmul(ps, lhsT, rhs, start=start, stop=stop)

    def f32(ap):
        return ap.bitcast(F32)

    with TileContext(nc) as tc:
        with (
            tc.tile_pool(name="const", bufs=1) as cpool,
            tc.tile_pool(name="ps", bufs=8, space="PSUM") as pspool,
            tc.tile_pool(name="rows", bufs=4) as rpool,
            tc.tile_pool(name="rbp", bufs=2) as rbpool,
            tc.tile_pool(name="work", bufs=4) as tpool,
            tc.tile_pool(name="wbig", bufs=3) as wpool,
            tc.tile_pool(name="big", bufs=1) as gpool,
        ):
          def emit_body():
            def ctile(shape, tag):
                return cpool.tile(shape, F32, tag=tag, name=tag)

            def wtile(name):
                return wpool.tile([128, 8, 256], mmdt, tag="w", name=name)

            # ---------- constants / small inputs ----------
            ones_col = ctile([128, 1], "ones_col")
            nc.vector.memset(ones_col[:], 1.0)
            ones_colr = cpool.tile([128, 1], mmdt, tag="ones_colr", name="ones_colr")
            nc.scalar.activation(ones_colr[:], ones_col[:], AF.Copy)
            ones_row = ctile([1, 128], "ones_row")
            nc.vector.memset(ones_row[:], 1.0)
            ones_rowr = cpool.tile([1, 128], mmdt, tag="ones_rowr", name="ones_rowr")
            nc.scalar.activation(ones_rowr[:], ones_row[:], AF.Copy)

            ln8 = ctile([128, 1], "ln8")
            nc.vector.memset(ln8[:], float(math.log(0.125)))
            epsrow = ctile([1, 1], "epsrow")
            nc.vector.memset(epsrow[:], EPS)

            svec = ctile([128, 8], "svec")
            nc.sync.dma_start(out=svec[:], in_=ltau_d[:])
            # svec = exp(-log_tau) / 8   (folds 1/sqrt(dh)=1/8 and 1/tau)
            nc.scalar.activation(svec[:], svec[:], AF.Exp,
                                 scale=-1.0, bias=ln8[:])

            small_loads = [("maskb", maskb_d, [128, 4]), ("clsw", clsw_d, [128, 8]),
                           ("g1", g1_d, [128, 8]), ("gb1", gb1_d, [128, 8]),
                           ("g2", g2_d, [128, 8]), ("gb2", gb2_d, [128, 8]),
                           ("bqp", bq_d, [128, 8]), ("bkp", bk_d, [128, 8]),
                           ("bvp", bv_d, [128, 8]), ("bop", bo_d, [128, 8]),
                           ("b1p", b1_d, [128, 32]), ("b2p", b2_d, [128, 8]),
                           ("cbr", cb_d, [1, 1])]
            sm = {}
            for nm, dram, shp in small_loads:
                t = ctile(shp, nm)
                nc.sync.dma_start(out=t[:], in_=dram[:])
                sm[nm] = t
            maskb, clsw = sm["maskb"], sm["clsw"]
            g1, gb1, g2, gb2 = sm["g1"], sm["gb1"], sm["g2"], sm["gb2"]
            bqp, bkp, bvp, bop = sm["bqp"], sm["bkp"], sm["bvp"], sm["bop"]
            b1p, b2p, cbr = sm["b1p"], sm["b2p"], sm["cbr"]
            clswr = cpool.tile([128, 8], mmdt, tag="clswr", name="clswr")
            nc.scalar.activation(clswr[:], clsw[:], AF.Copy)
            # q bias must be pre-scaled by svec (drain computes ps*svec + bias)
            bqs = ctile([128, 8], "bqs")
            nc.vector.tensor_mul(bqs[:], bqp[:], svec[:])

            def ln_transposed(xt, g_t, gb_t, xdt):
                """In-place LayerNorm over the feature axis (partition x
                po-chunk) of a [128, 8, 1024] transposed activation tile."""
                oc = ones_colr if xdt is not F32 else ones_col
                for pc in range(2):
                    pcs = slice(pc * N, (pc + 1) * N)
                    sum_ps = pspool.tile([128, N], F32, tag="ps", name="sum_ps")
                    for dc in range(8):
                        mm(sum_ps[0:1, :], oc[:], xt[:, dc, pcs],
                           start=(dc == 0), stop=(dc == 7))
                    sumsq_ps = pspool.tile([128, N], F32, tag="ps", name="sumsq_ps")
                    for dc in range(8):
                        sq = tpool.tile([128, N], xdt, tag="work", name="sq")
                        nc.scalar.activation(sq[:], f32(xt[:, dc, pcs]), AF.Square)
                        mm(sumsq_ps[0:1, :], oc[:], sq[:],
                           start=(dc == 0), stop=(dc == 7))
                    mu = rpool.tile([1, N], F32, tag="rows", name="mu")
                    nc.scalar.activation(mu[:], sum_ps[0:1, :], AF.Copy,
                                         scale=1.0 / D)
                    ex2 = rpool.tile([1, N], F32, tag="rows", name="ex2")
                    nc.scalar.activation(ex2[:], sumsq_ps[0:1, :], AF.Copy,
                                         scale=1.0 / D)
                    var = rpool.tile([1, N], F32, tag="rows", name="var")
                    nc.scalar.activation(var[:], mu[:], AF.Square)
                    nc.vector.tensor_sub(var[:], ex2[:], var[:])
                    std = rpool.tile([1, N], F32, tag="rows", name="std")
                    nc.scalar.activation(std[:], var[:], AF.Sqrt, bias=epsrow[:])
                    rstd = rpool.tile([1, N], F32, tag="rows", name="rstd")
                    nc.vector.reciprocal(rstd[:], std[:])
                    mu_bc = rbpool.tile([128, N], F32, tag="rbp", name="mu_bc")
                    nc.gpsimd.partition_broadcast(mu_bc[:], mu[:])
                    rstd_bc = rbpool.tile([128, N], F32, tag="rbp", name="rstd_bc")
                    nc.gpsimd.partition_broadcast(rstd_bc[:], rstd[:])
                    for dc in range(8):
                        t1 = tpool.tile([128, N], F32, tag="work", name="t1")
                        nc.vector.tensor_sub(t1[:], f32(xt[:, dc, pcs]), mu_bc[:])
                        nc.vector.tensor_mul(t1[:], t1[:], rstd_bc[:])
                        nc.vector.tensor_scalar(
                            out=xt[:, dc, pcs], in0=t1[:],
                            scalar1=g_t[:, dc:dc + 1], scalar2=gb_t[:, dc:dc + 1],
                            op0=ALU.mult, op1=ALU.add)

            # ================= Phase A: QKV projections =================
            imgT = gpool.tile([128, 8, P], mmdt, tag="bigA", name="imgT")
            nc.sync.dma_start(
                out=imgT[:], in_=imgT_d[:].rearrange("(po pi) p -> pi po p", pi=128))
            txtT = gpool.tile([128, 8, N], mmdt, tag="bigD", name="txtT", bufs=2)
            nc.sync.dma_start(
                out=txtT[:], in_=txtT_d[:].rearrange("(po pi) n -> pi po n", pi=128))

            qT = gpool.tile([128, 8, P], mmdt, tag="bigB", name="qT")
            kT = gpool.tile([128, 8, N], mmdt, tag="bigE", name="kT")
            v_sb = gpool.tile([128, 4, H * (DH + 1)], mmdt, tag="bigF", name="v_sb")

            # ones column for every head slot in v (denominator trick)
            nc.vector.tensor_copy(
                v_sb[:].rearrange("q n (h e) -> q n h e", e=DH + 1)[:, :, :, DH:DH + 1],
                ones_col[:].to_broadcast((128, 4, H, 1)))

            # qT = (Wq @ img^T + bq) scaled by svec (per-feature 1/(8*tau_h))
            for dq in range(4):
                dos = slice(256 * dq, 256 * (dq + 1))
                wq_q = wtile(f"wq{dq}")
                nc.sync.dma_start(
                    out=wq_q[:], in_=wqT_d[:, dos].rearrange("(po pi) d -> pi po d", pi=128))
                for dl in range(2):
                    doutc = 2 * dq + dl
                    for pc in range(2):
                        pcs = slice(pc * N, (pc + 1) * N)
                        ps = pspool.tile([128, N], F32, tag="ps", name="ps")
                        for dinc in range(8):
                            mm(ps[:], wq_q[:, dinc, 128 * dl:128 * (dl + 1)],
                               imgT[:, dinc, pcs], start=(dinc == 0), stop=(dinc == 7))
                        nc.scalar.activation(qT[:, doutc, pcs], ps[:], AF.Identity,
                                             scale=svec[:, doutc:doutc + 1],
                                             bias=bqs[:, doutc:doutc + 1])

            # kT = Wk @ txt^T + bk
            for dq in range(4):
                dos = slice(256 * dq, 256 * (dq + 1))
                wk_q = wtile(f"wk{dq}")
                nc.sync.dma_start(
                    out=wk_q[:], in_=wkT_d[:, dos].rearrange("(po pi) d -> pi po d", pi=128))
                for dl in range(2):
                    doutc = 2 * dq + dl
                    ps = pspool.tile([128, N], F32, tag="ps", name="ps")
                    for dinc in range(8):
                        mm(ps[:], wk_q[:, dinc, 128 * dl:128 * (dl + 1)],
                           txtT[:, dinc, :], start=(dinc == 0), stop=(dinc == 7))
                    nc.scalar.activation(kT[:, doutc, :], ps[:], AF.Identity,
                                         bias=bkp[:, doutc:doutc + 1])

            # v natural [n, dout], written into the strided head+1 layout
            for dq in range(4):
                wv_q = wtile(f"wv{dq}")
                nc.sync.dma_start(
                    out=wv_q[:], in_=wvT_d[:, 256 * dq:256 * (dq + 1)].rearrange(
                        "(po pi) d -> pi po d", pi=128))
                for nc4 in range(4):
                    ps = pspool.tile([128, 256], F32, tag="ps", name="ps")
                    for dinc in range(8):
                        mm(ps[:], txtT[:, dinc, 128 * nc4:128 * (nc4 + 1)],
                           wv_q[:, dinc, :], start=(dinc == 0), stop=(dinc == 7))
                    dst = v_sb[:].rearrange("q n (h e) -> q n h e", e=DH + 1)[
                        :, nc4, 4 * dq:4 * (dq + 1), 0:DH]
                    src = ps[:].rearrange("q (h d) -> q h d", d=DH)
                    nc.vector.tensor_copy(dst, src)

            # ================= Phase B: attention =================
            attnT = gpool.tile([128, 8, P], mmdt, tag="bigC", name="attnT")
            for h in range(H):
                hp = 64 * (h % 2)
                po = h // 2
                expT = gpool.tile([128, 4, P], mmdt, tag="bigD", name="expT", bufs=2)
                for pc in range(2):
                    pcs = slice(pc * N, (pc + 1) * N)
                    for nc4 in range(4):
                        sps = pspool.tile([128, N], F32, tag="ps", name="sps")
                        mm(sps[:],
                           kT[hp:hp + 64, po, 128 * nc4:128 * (nc4 + 1)],
                           qT[hp:hp + 64, po, pcs],
                           start=True, stop=True)
                        nc.scalar.activation(expT[:, nc4, pcs], sps[:], AF.Exp,
                                             bias=maskb[:, nc4:nc4 + 1])
                    aps = pspool.tile([128, N], F32, tag="ps", name="aps")
                    for nc4 in range(4):
                        mm(aps[0:DH + 1, :],
                           v_sb[:, nc4, (DH + 1) * h:(DH + 1) * (h + 1)],
                           expT[:, nc4, pcs],
                           start=(nc4 == 0), stop=(nc4 == 3))
                    r_row = rpool.tile([1, N], F32, tag="rows", name="r_row")
                    nc.vector.reciprocal(r_row[:], aps[DH:DH + 1, :])
                    rb = rbpool.tile([128, N], F32, tag="rbp", name="rb")
                    nc.gpsimd.partition_broadcast(rb[:], r_row[:])
                    # normalized attention output (transposed layout) + v bias
                    nc.vector.tensor_mul(attnT[hp:hp + 64, po, pcs],
                                         aps[0:DH, :], rb[0:DH, :])
                    nc.vector.tensor_scalar_add(attnT[hp:hp + 64, po, pcs],
                                                f32(attnT[hp:hp + 64, po, pcs]),
                                                bvp[hp:hp + 64, po:po + 1])
                    # normalized probabilities -> DRAM (transposed)
                    for nc4 in range(4):
                        pt = tpool.tile([128, N], F32, tag="work", name="pt")
                        nc.vector.tensor_mul(pt[:], f32(expT[:, nc4, pcs]), rb[:])
                        nc.sync.dma_start(
                            out=probsT_o[h, 128 * nc4:128 * (nc4 + 1), pcs],
                            in_=pt[:])

            # ============== Phase C: out-proj + LN1 ==============
            x1T = gpool.tile([128, 8, P], mmdt, tag="bigB", name="x1T")
            for dq in range(4):
                wo_q = wtile(f"wo{dq}")
                nc.sync.dma_start(
                    out=wo_q[:], in_=woT_d[:, 256 * dq:256 * (dq + 1)].rearrange(
                        "(po pi) d -> pi po d", pi=128))
                for dl in range(2):
                    doutc = 2 * dq + dl
                    for pc in range(2):
                        pcs = slice(pc * N, (pc + 1) * N)
                        ps = pspool.tile([128, N], F32, tag="ps", name="ps")
                        for dinc in range(8):
                            mm(ps[:], wo_q[:, dinc, 128 * dl:128 * (dl + 1)],
                               attnT[:, dinc, pcs],
                               start=(dinc == 0), stop=(dinc == 7))
                        nc.vector.scalar_tensor_tensor(
                            out=x1T[:, doutc, pcs], in0=ps[:],
                            scalar=bop[:, doutc:doutc + 1],
                            in1=f32(imgT[:, doutc, pcs]),
                            op0=ALU.add, op1=ALU.add)
            ln_transposed(x1T, g1, gb1, mmdt)

            # ============== Phase D: FFN + LN2 ==============
            xT_sb = gpool.tile([128, 8, P], mmdt, tag="bigC", name="xT_sb")
            hT_a = gpool.tile([128, 16, N], mmdt, tag="bigA", name="hT_a")
            hT_b = gpool.tile([128, 8, N], mmdt, tag="bigE", name="hT_b")
            hT_c = gpool.tile([128, 8, N], mmdt, tag="bigF", name="hT_c")

            def h_slot(hc):
                if hc < 16:
                    return hT_a[:, hc, :]
                if hc < 24:
                    return hT_b[:, hc - 16, :]
                return hT_c[:, hc - 24, :]

            for pc in range(2):
                pcs = slice(pc * N, (pc + 1) * N)
                # D1: hT = relu(W1 @ x1^T + b1) for this p-half
                for hc in range(32):
                    w1s = wpool.tile([128, 8, 128], mmdt, tag="w", name="w1s")
                    nc.sync.dma_start(
                        out=w1s[:],
                        in_=w1T_d[:, 128 * hc:128 * (hc + 1)].rearrange(
                            "(po pi) hh -> pi po hh", pi=128))
                    ps = pspool.tile([128, N], F32, tag="ps", name="ps")
                    for dinc in range(8):
                        mm(ps[:], w1s[:, dinc, :], x1T[:, dinc, pcs],
                           start=(dinc == 0), stop=(dinc == 7))
                    nc.scalar.activation(h_slot(hc), ps[:], AF.Relu,
                                         bias=b1p[:, hc:hc + 1])
                # D2: x2 = W2 @ hT + b2 (+x1 residual)
                aps2 = [pspool.tile([128, N], F32, tag="ps", name=f"acc{i}")
                        for i in range(8)]
                for hc in range(32):
                    w2s = wpool.tile([128, 1024], mmdt, tag="w", name="w2s")
                    nc.sync.dma_start(
                        out=w2s[:], in_=w2T_d[128 * hc:128 * (hc + 1), :])
                    for dc in range(8):
                        mm(aps2[dc][:], w2s[:, 128 * dc:128 * (dc + 1)],
                           h_slot(hc), start=(hc == 0), stop=(hc == 31))
                for dc in range(8):
                    nc.vector.scalar_tensor_tensor(
                        out=xT_sb[:, dc, pcs], in0=aps2[dc][:],
                        scalar=b2p[:, dc:dc + 1], in1=f32(x1T[:, dc, pcs]),
                        op0=ALU.add, op1=ALU.add)
            ln_transposed(xT_sb, g2, gb2, mmdt)
            for dc in range(8):
                nc.sync.dma_start(out=xT_o[128 * dc:128 * (dc + 1), :],
                                  in_=f32(xT_sb[:, dc, :]))

            # ============== Phase E: classifier head ==============
            for pc in range(2):
                pcs = slice(pc * N, (pc + 1) * N)
                lp = pspool.tile([128, N], F32, tag="ps", name="lp")
                for dc in range(8):
                    mm(lp[0:1, :], clswr[:, dc:dc + 1], xT_sb[:, dc, pcs],
                       start=(dc == 0), stop=(dc == 7))
                lrow = tpool.tile([128, N], F32, tag="work", name="lrow")
                nc.scalar.activation(lrow[0:1, :], lp[0:1, :], AF.Identity,
                                     bias=cbr[:])
                srow = tpool.tile([128, N], F32, tag="work", name="srow")
                nc.scalar.activation(srow[0:1, :], lrow[0:1, :], AF.Sigmoid)
                nc.sync.dma_start(out=logits_o[0:1, pcs], in_=lrow[0:1, :])
                nc.sync.dma_start(out=sig_o[0:1, pcs], in_=srow[0:1, :])

          if repeat == 1:
              emit_body()
          else:
              with tc.For_i(0, repeat, 1):
                  emit_body()

    nc.finalize()
    return nc


_NC_CACHE = {}


def _get_program(repeat=1):
    key = (_MMDT_NAME, repeat)
    if key not in _NC_CACHE:
        _NC_CACHE[key] = _build_program(_MMDT_NAME, repeat)
    return _NC_CACHE[key]


def make_in_maps(img_emb, txt_emb, text_mask, in_proj_w, in_proj_b, out_w, out_b,
                 log_tau, n1_g, n1_b, ffn_w1, ffn_b1, ffn_w2, ffn_b2, n2_g, n2_b,
                 cls_w, cls_b):
    """Host-side marshaling: shard over batch and lay tensors out as the
    device program expects (all pure transpose/reshape/replication)."""
    f = np.float32
    img_emb = np.asarray(img_emb, f)
    txt_emb = np.asarray(txt_emb, f)
    text_mask = np.asarray(text_mask)
    in_proj_w = np.asarray(in_proj_w, f)
    in_proj_b = np.asarray(in_proj_b, f)

    def pp(vec, cols):
        return np.ascontiguousarray(np.asarray(vec, f).reshape(cols, 128).T)

    shared = {
        "wqT": np.ascontiguousarray(in_proj_w[:D].T),
        "wkT": np.ascontiguousarray(in_proj_w[D:2 * D].T),
        "wvT": np.ascontiguousarray(in_proj_w[2 * D:].T),
        "woT": np.ascontiguousarray(np.asarray(out_w, f).T),
        "w1T": np.ascontiguousarray(np.asarray(ffn_w1, f).T),
        "w2T": np.ascontiguousarray(np.asarray(ffn_w2, f).T),
        "bq": pp(in_proj_b[:D], 8),
        "bk": pp(in_proj_b[D:2 * D], 8),
        "bv": pp(in_proj_b[2 * D:], 8),
        "bo": pp(out_b, 8),
        "b1": pp(ffn_b1, 32),
        "b2": pp(ffn_b2, 8),
        "cb": np.asarray(cls_b, f).reshape(1, 1),
        "ltau": np.ascontiguousarray(
            np.repeat(np.asarray(log_tau, f).reshape(8, 2).T, 64, axis=0)),
        "clsw": pp(np.asarray(cls_w, f)[0], 8),
        "g1": pp(n1_g, 8),
        "gb1": pp(n1_b, 8),
        "g2": pp(n2_g, 8),
        "gb2": pp(n2_b, 8),
    }
    in_maps = []
    for b in range(B):
        mb = np.where(text_mask[b], 0.0, -1e30).astype(f)
        m = dict(shared)
        m["imgT"] = np.ascontiguousarray(img_emb[b].T)
        m["txtT"] = np.ascontiguousarray(txt_emb[b].T)
        m["maskb"] = np.ascontiguousarray(mb.reshape(4, 128).T)
        in_maps.append(m)
    return in_maps


def unshard(results):
    """Gather per-core outputs into full-shape numpy arrays (host-side
    transpose undoes the device's feature-major layout)."""
    x = np.stack([np.ascontiguousarray(r["xT_o"].T) for r in results])
    weights = np.stack(
        [np.ascontiguousarray(r["probsT_o"].transpose(0, 2, 1)) for r in results])
    logits = np.stack([r["logits_o"][0] for r in results])
    sig = np.stack([r["sig_o"][0] for r in results])
    return x, weights, logits, sig


def kernel(**inputs):
    nc = _get_program()
    in_maps = make_in_maps(**inputs)
    res = run_bass_kernel_spmd(nc, in_maps, list(range(N_CORES)))
    return unshard(res.results)
